# revision 9
# baseline (speedup 1.0000x reference)
"""Node2VecHypergraphConv distributed Trainium2 kernel v2 (8 NeuronCores).

Algorithm (reference):
    x = emb @ conv_w.T
    e = Binv * segsum_edge(x[node_idx])          # node -> hyperedge
    n = Dinv * segsum_node(e[edge_idx]) + conv_b # hyperedge -> node
    y = lrelu(n); g = y.T @ y
    out = lrelu(g @ lin_w.T + lin_b)

v2 design:
    Phase A consumes a HOST-pregathered fp8 stream of emb rows (one
    contiguous DMA per chunk group — no per-incidence descriptors),
    scatter-summed into per-edge-window PSUM via one-hot fp8 DoubleRow
    matmuls, W applied per window, e rows stored fp8. The e table is
    AllGathered in 2 segments so phase-B gathers (the only per-incidence
    DMA left) start while phase A is still running. Phase B runs two
    passes (seg0 partials stashed in SBUF via the Act engine) so the
    seg0 gather stream never blocks on seg1 availability. y finalize on
    Act+DVE, Gram accumulated in PSUM bf16, AllReduce, tiny final matmul.
"""
import os
import sys

sys.path.insert(0, '/opt/trn_rl_repo')
import numpy as np

NCORES = 8
N_NODES = 50000
N_EDGES = 10000
C = 256
NEG = 0.01
E_PER = N_EDGES // NCORES          # 1250
N_PER = N_NODES // NCORES          # 6250
NW_A = -(-E_PER // 128)            # 10
NW_B = -(-N_PER // 128)            # 49
SEG_W = int(os.environ.get('K2_SEG_W', '5'))  # phase-A windows in AG seg 0
SEG_ROWS = (SEG_W * 128, E_PER - SEG_W * 128)   # (640, 610)
GA = int(os.environ.get('K2_GA', '32'))         # phase-A chunks per load group
IPG = int(os.environ.get('K2_IPG', '1024'))     # phase-B idx per dma_gather
NQ = 4
AG1_FRAC = float(os.environ.get('K2_AG1_FRAC', '0.5'))
SP = os.environ.get('K2_SP', '1') == '1'  # dma_gather single_packet


def _ceil(a, b):
    return -(-a // b)


def _wrap_idx(a):
    """int16 index vector -> dma_gather SBUF layout [128, L/16]."""
    L = a.shape[0]
    assert L % 16 == 0
    w = a.reshape(L // 16, 16).T.astype(np.int16)
    return np.ascontiguousarray(np.tile(w, (8, 1)))


def _cols(v, nch, ncp):
    """per-slot values [nch*128] -> [128, ncp] f32, pad cols -1."""
    out = np.full((128, ncp), -1.0, dtype=np.float32)
    if nch:
        out[:, :nch] = v[:nch * 128].reshape(nch, 128).T
    return out


def preprocess(edge_index):
    node_idx = np.asarray(edge_index[0], dtype=np.int64)
    edge_idx = np.asarray(edge_index[1], dtype=np.int64)
    nnz = node_idx.shape[0]

    D = np.bincount(node_idx, minlength=N_NODES).astype(np.float32)
    B = np.bincount(edge_idx, minlength=N_EDGES).astype(np.float32)
    Dinv = np.where(D > 0, 1.0 / np.maximum(D, 1.0), 0.0).astype(np.float32)
    Binv = np.where(B > 0, 1.0 / np.maximum(B, 1.0), 0.0).astype(np.float32)

    # ---------------- phase A buckets: (edge core, window) ----------------
    core_a = edge_idx // E_PER
    eloc = edge_idx - core_a * E_PER
    win_a = eloc >> 7
    ecol_a = (eloc & 127).astype(np.float32)
    cnt = np.zeros((NCORES, NW_A), np.int64)
    np.add.at(cnt, (core_a, win_a), 1)
    M_a = np.array([_ceil(int(cnt[:, w].max()), 128) for w in range(NW_A)])
    base_a = np.concatenate([[0], np.cumsum(M_a[:-1])]) * 128
    NCHA = int(M_a.sum())
    NCHA_P = _ceil(NCHA, GA) * GA

    order = np.lexsort((win_a, core_a))
    oc, ow = core_a[order], win_a[order]
    onode, oecol = node_idx[order], ecol_a[order]
    key = oc * NW_A + ow
    starts = np.flatnonzero(np.concatenate([[True], key[1:] != key[:-1]]))
    rank = np.arange(nnz) - np.repeat(starts, np.diff(np.concatenate([starts, [nnz]])))

    # ---------------- phase B buckets: (node core, window, src seg) --------
    core_b = node_idx // N_PER
    nloc = node_idx - core_b * N_PER
    win_b = nloc >> 7
    ncol_b = (nloc & 127).astype(np.float32)
    seg = (eloc >= SEG_ROWS[0]).astype(np.int64)
    # dedupe: one gather slot per distinct (core, window, seg, edge); the
    # host-built S column carries the multiplicity.
    keyu = ((core_b * 2 + seg) * NW_B + win_b) * N_EDGES + edge_idx
    uniq, inv = np.unique(keyu, return_inverse=True)
    u_edge = uniq % N_EDGES
    u_bucket = uniq // N_EDGES
    u_win = u_bucket % NW_B
    u_seg = (u_bucket // NW_B) % 2
    u_core = u_bucket // (2 * NW_B)
    u_ecore = u_edge // E_PER
    u_eloc = u_edge - u_ecore * E_PER
    u_gidx = np.where(u_seg == 0, u_ecore * SEG_ROWS[0] + u_eloc,
                      u_ecore * SEG_ROWS[1] + (u_eloc - SEG_ROWS[0]))
    stu = np.flatnonzero(np.concatenate([[True], u_bucket[1:] != u_bucket[:-1]]))
    nuniq = len(uniq)
    u_rank = np.arange(nuniq) - np.repeat(
        stu, np.diff(np.concatenate([stu, [nuniq]])))
    ucnt = np.zeros((NCORES, NW_B, 2), np.int64)
    bid = u_bucket[stu]
    ucnt[bid // (2 * NW_B), bid % NW_B, (bid // NW_B) % 2] = \
        np.diff(np.concatenate([stu, [nuniq]]))
    M_b = np.array([[_ceil(int(ucnt[:, w, s].max()), 128) for s in range(2)]
                    for w in range(NW_B)])
    NCHB = [int(M_b[:, s].sum()) for s in range(2)]
    L_b = [n * 128 for n in NCHB]
    LP_b = [_ceil(max(L, 1), IPG) * IPG for L in L_b]
    ncp_b = [LP // 128 for LP in LP_b]
    base_b = np.zeros((NW_B, 2), np.int64)
    for s in range(2):
        base_b[:, s] = np.concatenate([[0], np.cumsum(M_b[:-1, s])]) * 128

    anode = np.zeros((NCORES, NCHA_P * 128), np.int64)
    acol = [None] * NCORES
    idx_b = [[None, None] for _ in range(NCORES)]
    sbmat = [[None, None] for _ in range(NCORES)]
    u_slot_all = base_b[u_win, u_seg] + u_rank
    inc_slot = u_slot_all[inv]
    ncol_i = (nloc & 127).astype(np.int64)
    import ml_dtypes
    for c in range(NCORES):
        sel = oc == c
        slot = base_a[ow[sel]] + rank[sel]
        av = np.full(NCHA_P * 128, -1.0, np.float32)
        anode[c][slot] = onode[sel]
        av[slot] = oecol[sel]
        acol[c] = _cols(av, NCHA_P, NCHA_P)
        for s in range(2):
            selu = (u_core == c) & (u_seg == s)
            gi = np.zeros(LP_b[s], np.int64)
            gi[u_slot_all[selu]] = u_gidx[selu]
            idx_b[c][s] = _wrap_idx(gi.astype(np.int16))
            seli = (core_b == c) & (seg == s)
            S = np.zeros((ncp_b[s] * 128, 128), np.float32)
            np.add.at(S, (inc_slot[seli], ncol_i[seli]), 1.0)
            sbmat[c][s] = np.ascontiguousarray(
                S.reshape(ncp_b[s], 128, 128).transpose(1, 0, 2)
            ).astype(ml_dtypes.float8_e4m3)

    binv_cols = np.zeros((NCORES, 128, NW_A), np.float32)
    dinv_cols = np.zeros((NCORES, 128, NW_B), np.float32)
    mask_cols = np.zeros((NCORES, 128, NW_B), np.float32)
    for c in range(NCORES):
        bv = np.pad(Binv[c * E_PER:(c + 1) * E_PER], (0, NW_A * 128 - E_PER))
        binv_cols[c] = bv.reshape(NW_A, 128).T
        dv = np.pad(Dinv[c * N_PER:(c + 1) * N_PER], (0, NW_B * 128 - N_PER))
        dinv_cols[c] = dv.reshape(NW_B, 128).T
        mk = np.pad(np.ones(N_PER, np.float32), (0, NW_B * 128 - N_PER))
        mask_cols[c] = mk.reshape(NW_B, 128).T

    meta = dict(M_a=M_a, M_b=M_b, NCHA_P=NCHA_P, NCHB=NCHB,
                L_b=L_b, LP_b=LP_b, ncp_b=ncp_b)
    percore = dict(anode=anode, acol=acol, idx_b=idx_b, sbmat=sbmat,
                   binv_cols=binv_cols, dinv_cols=dinv_cols,
                   mask_cols=mask_cols)
    return meta, percore


def build_kernel(meta, has_cb):
    import concourse.bacc as bacc
    import concourse.mybir as mybir
    import concourse.tile as tile

    f32 = mybir.dt.float32
    bf16 = mybir.dt.bfloat16
    fp8 = mybir.dt.float8e4
    i16 = mybir.dt.int16
    DR = mybir.MatmulPerfMode.DoubleRow
    EQ = mybir.AluOpType.is_equal

    M_a, M_b = meta['M_a'], meta['M_b']
    NCHA_P = meta['NCHA_P']
    LP_b, ncp_b = meta['LP_b'], meta['ncp_b']
    NGA = NCHA_P // GA
    GC = IPG // 128
    n_g = [LP // IPG for LP in LP_b]
    groups = [list(range(NCORES))]

    nc = bacc.Bacc('TRN2', num_devices=NCORES,
                   dynamic_dma_scratch_size=65536, num_swdge_queues=NQ)

    astream = nc.declare_dram_parameter("astream", [128, NCHA_P, C], fp8,
                                        isOutput=False)
    p_acol = nc.declare_dram_parameter("acol", [128, NCHA_P], f32, isOutput=False)
    p_idx = [nc.declare_dram_parameter(f"idx_b{s}", [128, LP_b[s] // 16], i16,
                                       isOutput=False) for s in range(2)]
    p_sb = [nc.declare_dram_parameter(f"sb{s}", [128, ncp_b[s], 128], fp8,
                                      isOutput=False) for s in range(2)]
    p_binv = nc.declare_dram_parameter("binv_cols", [128, NW_A], f32, isOutput=False)
    p_dinv = nc.declare_dram_parameter("dinv_cols", [128, NW_B], f32, isOutput=False)
    p_dinv2 = nc.declare_dram_parameter("dinv2_cols", [128, NW_B], f32,
                                        isOutput=False)
    p_mask = nc.declare_dram_parameter("mask_cols", [128, NW_B], f32, isOutput=False)
    p_wtb = nc.declare_dram_parameter("wtb", [128, 2, C], bf16, isOutput=False)
    p_lwt = nc.declare_dram_parameter("lwt", [128, 2, C], f32, isOutput=False)
    p_cb = nc.declare_dram_parameter("convb_bc", [128, C], f32, isOutput=False)
    p_lb = nc.declare_dram_parameter("linb_bc", [128, C], f32, isOutput=False)
    p_iota = nc.declare_dram_parameter("iota", [128, 128], f32, isOutput=False)
    p_identb = nc.declare_dram_parameter("identb", [128, 128], bf16, isOutput=False)
    out = nc.declare_dram_parameter("out", [C, C], f32, isOutput=True)

    gq = [0]
    with tile.TileContext(nc) as tc:
        with (
            tc.tile_pool(name="dram", bufs=1, space="DRAM") as dram,
            tc.tile_pool(name="const", bufs=1) as constp,
            tc.tile_pool(name="idx", bufs=1) as idxp,
            tc.tile_pool(name="stash", bufs=1) as stashp,
            tc.tile_pool(name="sb", bufs=6) as sbp,
        ):
            agin = [dram.tile([SEG_ROWS[s], C], fp8, name=f"agin{s}")
                    for s in range(2)]
            efull = [dram.tile([SEG_ROWS[s] * NCORES, C], fp8,
                               addr_space="Shared", name=f"efull{s}")
                     for s in range(2)]
            arin = dram.tile([128, 2, C], f32)
            gfull = dram.tile([128, 2, C], f32, addr_space="Shared")

            iota = constp.tile([128, 128], f32)
            identb = constp.tile([128, 128], bf16)
            wtb = constp.tile([128, 2, C], bf16)
            lwt = constp.tile([128, 2, C], f32)
            cb = constp.tile([128, C], f32)
            lb = constp.tile([128, C], f32)
            binv = constp.tile([128, NW_A], f32)
            dinv = constp.tile([128, NW_B], f32)
            dinv2 = constp.tile([128, NW_B], f32)
            mask = constp.tile([128, NW_B], f32)
            acol = idxp.tile([128, NCHA_P], f32)
            idxb = [idxp.tile([128, LP_b[s] // 16], i16, name=f"idxb{s}")
                    for s in range(2)]
            for dst, src in ((iota, p_iota), (identb, p_identb), (wtb, p_wtb),
                             (lwt, p_lwt), (cb, p_cb), (lb, p_lb),
                             (binv, p_binv), (dinv, p_dinv),
                             (dinv2, p_dinv2), (mask, p_mask),
                             (acol, p_acol),
                             (idxb[0], p_idx[0]), (idxb[1], p_idx[1])):
                nc.sync.dma_start(dst[:], src[:])

            v0all = stashp.tile([128, NW_B, C], bf16)

            SBG = 4
            sbt = [[], []]
            sload = [0]

            def load_s_tiles(s, lo_t, hi_t):
                ng_s = LP_b[s] // IPG
                for tix in range(lo_t, hi_t):
                    g = tix * SBG
                    span = min(SBG, ng_s - g)
                    sf = sbp.tile([128, SBG * GC, 128], fp8, tag=f"sb{s}",
                                  name=f"sb{s}_{g}")
                    sload[0] += 1
                    # sync only: an S load blocked on pool space must never
                    # stall Act compute (deadlock via psB<->finalize order)
                    nc.sync.dma_start(sf[:, :span * GC, :],
                                      p_sb[s][:, g * GC:(g + span) * GC, :])
                    sbt[s].append(sf)

            # early runway: first S tiles of BOTH segs load before phase A
            n_sb = [_ceil(LP_b[s] // IPG, SBG) for s in range(2)]
            load_s_tiles(0, 0, min(2, n_sb[0]))
            load_s_tiles(1, 0, min(2, n_sb[1]))

            # ======================= PHASE A =======================
            with (
                tc.tile_pool(name="st", bufs=4) as stp,
                tc.tile_pool(name="sa", bufs=4) as sap,
                tc.tile_pool(name="psA", bufs=2, space="PSUM") as psA,
                tc.tile_pool(name="psT", bufs=2, space="PSUM") as psT,
                tc.tile_pool(name="epA", bufs=3) as epp,
            ):
                iota_bA = iota.rearrange("p (c j) -> p c j", c=1) \
                    .broadcast_to([128, GA, 128])
                st_tiles, sa_tiles = [], []
                for g in range(NGA):
                    st = stp.tile([128, GA, C], fp8, tag="st", name=f"st{g}")
                    nc.sync.dma_start(st[:], astream[:, g * GA:(g + 1) * GA, :])
                    sa = sap.tile([128, GA, 128], fp8, tag="sa", name=f"sa{g}")
                    ecb = acol[:, g * GA:(g + 1) * GA] \
                        .rearrange("p (c j) -> p c j", j=1) \
                        .broadcast_to([128, GA, 128])
                    nc.vector.tensor_tensor(sa[:], iota_bA, ecb, EQ)
                    st_tiles.append(st)
                    sa_tiles.append(sa)

                cpos = 0
                for w in range(NW_A):
                    eacc = psA.tile([128, C], f32, tag="eacc", name=f"eacc{w}")
                    n_ch = int(M_a[w])
                    done = 0
                    while done < n_ch:
                        g, slot = divmod(cpos, GA)
                        if done + 1 < n_ch and slot + 1 < GA:
                            nc.tensor.matmul(
                                eacc[:], sa_tiles[g][:, slot:slot + 2, :],
                                st_tiles[g][:, slot:slot + 2, :],
                                start=(done == 0), stop=(done + 2 == n_ch),
                                perf_mode=DR)
                            cpos += 2
                            done += 2
                        else:
                            nc.tensor.matmul(
                                eacc[:], sa_tiles[g][:, slot, :],
                                st_tiles[g][:, slot, :],
                                start=(done == 0), stop=(done + 1 == n_ch))
                            cpos += 1
                            done += 1
                    # epilogue: Binv scale (Act), transpose, W, fp8 e rows
                    ep = epp.tile([128, C], bf16, tag="ep", name=f"ep{w}")
                    nc.scalar.mul(ep[:], eacc[:], binv[:, w:w + 1])
                    ept = epp.tile([128, 2, 128], bf16, tag="ept", name=f"ept{w}")
                    for ks in range(2):
                        tp = psT.tile([128, 128], bf16, tag="tp", name=f"tp{w}_{ks}")
                        nc.tensor.transpose(tp[:], ep[:, ks * 128:(ks + 1) * 128],
                                            identb[:])
                        nc.scalar.copy(ept[:, ks, :], tp[:])
                    epm = psT.tile([128, C], f32, tag="epm", name=f"epm{w}")
                    for ks in range(2):
                        nc.tensor.matmul(epm[:], ept[:, ks, :], wtb[:, ks, :],
                                         start=(ks == 0), stop=(ks == 1))
                    esb = epp.tile([128, C], fp8, tag="esb", name=f"esb{w}")
                    nc.scalar.copy(esb[:], epm[:])
                    s = 0 if w < SEG_W else 1
                    r0 = w * 128 - s * SEG_ROWS[0]
                    nrow = min(128, SEG_ROWS[s] - r0)
                    # seg0 stores ride gpsimd (ahead of the gathers); seg1
                    # stores must NOT block the gather stream on that queue.
                    seng = nc.gpsimd if s == 0 else nc.sync
                    seng.dma_start(agin[s][r0:r0 + nrow, :], esb[:nrow, :])
                    if w == SEG_W - 1:
                        nc.gpsimd.collective_compute(
                            "AllGather", mybir.AluOpType.bypass,
                            replica_groups=groups,
                            ins=[agin[0][:]], outs=[efull[0][:]])

            # ======================= PHASE B =======================
            with (
                tc.tile_pool(name="gb0", bufs=8) as gbp0,
                tc.tile_pool(name="gb1", bufs=8) as gbp1,
                tc.tile_pool(name="psB", bufs=2, space="PSUM") as psB,
                tc.tile_pool(name="psB1", bufs=3, space="PSUM") as psB1,
                tc.tile_pool(name="psG", bufs=1, space="PSUM") as psG,
                tc.tile_pool(name="yB", bufs=2) as yp,
                tc.tile_pool(name="fin", bufs=1) as finp,
            ):
                gt = [[], []]

                def issue_gathers(s, lo, hi):
                    pool = gbp0 if s == 0 else gbp1
                    for g in range(lo, hi):
                        t = pool.tile([128, GC, C], fp8, tag=f"gt{s}",
                                      name=f"gt{s}_{g}")
                        nc.gpsimd.dma_gather(
                            t[:], efull[s][:],
                            idxb[s][:, g * (IPG // 16):(g + 1) * (IPG // 16)],
                            IPG, IPG, C, queue_num=gq[0] % NQ,
                            single_packet=SP)
                        gq[0] += 1
                        gt[s].append(t)
                        if g % SBG == 0 and g // SBG >= 2:
                            load_s_tiles(s, g // SBG, g // SBG + 1)

                n_head = min(n_g[0], max(1, int(os.environ.get('K2_AGH', '12'))))
                issue_gathers(0, 0, n_head)
                nc.gpsimd.collective_compute(
                    "AllGather", mybir.AluOpType.bypass, replica_groups=groups,
                    ins=[agin[1][:]], outs=[efull[1][:]])
                r0, r1 = n_g[0] - n_head, n_g[1]
                i0 = i1 = 0
                while i0 < r0 or i1 < r1:
                    if i0 < r0 and (i1 >= r1 or i0 * r1 <= i1 * r0):
                        issue_gathers(0, n_head + i0, n_head + i0 + 1)
                        i0 += 1
                    else:
                        issue_gathers(1, i1, i1 + 1)
                        i1 += 1

                g_ps = [psG.tile([128, C], f32, tag=f"g{h}", name=f"g_ps{h}")
                        for h in range(2)]

                def seg_window(w, s, first, last_stop):
                    pool = psB if s == 0 else psB1
                    nacc = pool.tile([128, C], f32, tag=f"nacc{s}",
                                     name=f"nacc{s}_{w}")
                    n_ch = int(M_b[w][s])
                    ds = 0
                    while ds < n_ch:
                        g, slot = divmod(cposs[s], GC)
                        sg, soff = divmod(cposs[s], SBG * GC)
                        if ds + 1 < n_ch and slot + 1 < GC:
                            nc.tensor.matmul(
                                nacc[:], sbt[s][sg][:, soff:soff + 2, :],
                                gt[s][g][:, slot:slot + 2, :],
                                start=(ds == 0), stop=(ds + 2 == n_ch),
                                perf_mode=DR)
                            cposs[s] += 2
                            ds += 2
                        else:
                            nc.tensor.matmul(
                                nacc[:], sbt[s][sg][:, soff, :],
                                gt[s][g][:, slot, :],
                                start=(ds == 0), stop=(ds + 1 == n_ch))
                            cposs[s] += 1
                            ds += 1
                    return nacc

                cposs = [0, 0]
                KOFF = 16  # pass-2 trails pass-1 by KOFF windows
                KB = 7
                tbats = []

                def p2_window(w):
                    nacc = seg_window(w, 1, True, True)
                    if not has_cb:
                        b, k = divmod(w, KB)
                        if k == 0:
                            span = min(KB, NW_B - w)
                            tb = yp.tile([128, KB, C], bf16, tag="tb",
                                         name=f"tb{b}")
                            tbats.append((tb, w, span))
                        tb, w0, span = tbats[-1]
                        k = w - w0
                        # t = dinv-free partial sum: nacc1 + stashed nacc0
                        nc.vector.tensor_tensor(tb[:, k, :], nacc[:],
                                                v0all[:, w, :],
                                                mybir.AluOpType.add)
                        if k == span - 1:
                            ub = yp.tile([128, KB, C], bf16, tag="ub",
                                         name=f"ub{w0}")
                            nc.vector.tensor_scalar(
                                ub[:, :span, :], tb[:, :span, :], NEG, None,
                                mybir.AluOpType.mult)
                            nc.vector.tensor_tensor(
                                tb[:, :span, :], tb[:, :span, :],
                                ub[:, :span, :], mybir.AluOpType.max)
                            for kk in range(span):
                                ww = w0 + kk
                                ysc = yp.tile([128, C], bf16, tag="ysc",
                                              name=f"ysc{ww}")
                                nc.scalar.mul(ysc[:], tb[:, kk, :],
                                              dinv2[:, ww:ww + 1])
                                for hh in range(2):
                                    nc.tensor.matmul(
                                        g_ps[hh][:],
                                        ysc[:, hh * 128:(hh + 1) * 128],
                                        tb[:, kk, :],
                                        start=(ww == 0), stop=(ww == NW_B - 1))
                    else:
                        t = yp.tile([128, C], bf16, tag="t", name=f"t{w}")
                        nc.vector.tensor_tensor(t[:], nacc[:], v0all[:, w, :],
                                                mybir.AluOpType.add)
                        t2 = yp.tile([128, C], bf16, tag="t2", name=f"t2{w}")
                        nc.scalar.mul(t2[:], t[:], dinv[:, w:w + 1])
                        nc.vector.tensor_tensor(t2[:], t2[:], cb[:],
                                                mybir.AluOpType.add)
                        nc.vector.tensor_scalar(t2[:], t2[:], mask[:, w:w + 1],
                                                None, mybir.AluOpType.mult)
                        u = yp.tile([128, C], bf16, tag="u", name=f"u{w}")
                        nc.vector.tensor_scalar(u[:], t2[:], NEG, None,
                                                mybir.AluOpType.mult)
                        y = yp.tile([128, C], bf16, tag="y", name=f"y{w}")
                        nc.vector.tensor_tensor(y[:], t2[:], u[:],
                                                mybir.AluOpType.max)
                        for hh in range(2):
                            nc.tensor.matmul(
                                g_ps[hh][:], y[:, hh * 128:(hh + 1) * 128],
                                y[:], start=(w == 0), stop=(w == NW_B - 1))

                for w in range(NW_B + KOFF):
                    if w < NW_B:
                        nacc0 = seg_window(w, 0, True, True)
                        nc.scalar.copy(v0all[:, w, :], nacc0[:])
                    if w >= KOFF:
                        p2_window(w - KOFF)

                gsb = finp.tile([128, 2, C], f32)
                for hh in range(2):
                    nc.vector.tensor_copy(gsb[:, hh, :], g_ps[hh][:])
                nc.sync.dma_start(arin[:], gsb[:])
                nc.gpsimd.collective_compute(
                    "AllReduce", mybir.AluOpType.add, replica_groups=groups,
                    ins=[arin[:]], outs=[gfull[:]])
                gk = finp.tile([128, 2, C], f32)
                nc.sync.dma_start(gk[:], gfull[:])
                osb = finp.tile([128, 2, C], f32)
                for ih in range(2):
                    op = psB.tile([128, C], f32, tag="nacc0", name=f"ops{ih}")
                    for ks in range(2):
                        nc.tensor.matmul(
                            op[:], gk[:, ks, ih * 128:(ih + 1) * 128],
                            lwt[:, ks, :], start=(ks == 0), stop=(ks == 1))
                    tt = finp.tile([128, C], f32, tag=f"fin{ih}")
                    nc.vector.tensor_tensor(tt[:], op[:], lb[:],
                                            mybir.AluOpType.add)
                    uu = finp.tile([128, C], f32, tag=f"finu{ih}")
                    nc.vector.tensor_scalar(uu[:], tt[:], NEG, None,
                                            mybir.AluOpType.mult)
                    nc.vector.tensor_tensor(osb[:, ih, :], tt[:], uu[:],
                                            mybir.AluOpType.max)
                nc.sync.dma_start(out.rearrange("(h p) c -> p h c", h=2), osb[:])

    nc.compile()
    return nc


def make_in_maps(inputs, meta, percore):
    import ml_dtypes
    fp8 = ml_dtypes.float8_e4m3
    emb = np.asarray(inputs['emb'], dtype=np.float32).astype(fp8)
    conv_w = np.asarray(inputs['conv_w'], dtype=np.float32)
    conv_b = np.asarray(inputs['conv_b'], dtype=np.float32)
    lin_w = np.asarray(inputs['lin_w'], dtype=np.float32)
    lin_b = np.asarray(inputs['lin_b'], dtype=np.float32)

    wtb = np.ascontiguousarray(
        conv_w.T.reshape(2, 128, C).transpose(1, 0, 2)).astype(ml_dtypes.bfloat16)
    lwt = np.ascontiguousarray(
        lin_w.T.reshape(2, 128, C).transpose(1, 0, 2)).astype(np.float32)
    cb = np.ascontiguousarray(np.broadcast_to(conv_b, (128, C))).astype(np.float32)
    lb = np.ascontiguousarray(np.broadcast_to(lin_b, (128, C))).astype(np.float32)
    iota = np.ascontiguousarray(
        np.broadcast_to(np.arange(128, dtype=np.float32), (128, 128)))
    identb = np.eye(128, dtype=np.float32).astype(ml_dtypes.bfloat16)

    NCHA_P = meta['NCHA_P']
    in_maps = []
    for c in range(NCORES):
        stream = emb[percore['anode'][c]]                     # [NCHA_P*128, C]
        stream = np.ascontiguousarray(
            stream.reshape(NCHA_P, 128, C).transpose(1, 0, 2))  # [128, NCHA_P, C]
        in_maps.append(dict(
            astream=stream,
            acol=percore['acol'][c],
            idx_b0=percore['idx_b'][c][0], idx_b1=percore['idx_b'][c][1],
            sb0=percore['sbmat'][c][0], sb1=percore['sbmat'][c][1],
            binv_cols=percore['binv_cols'][c],
            dinv_cols=percore['dinv_cols'][c],
            dinv2_cols=percore['dinv_cols'][c] ** 2,
            mask_cols=percore['mask_cols'][c],
            wtb=wtb, lwt=lwt, convb_bc=cb, linb_bc=lb,
            iota=iota, identb=identb,
        ))
    return in_maps


def run(inputs, trace=False):
    from concourse.bass_utils import run_bass_kernel_spmd
    meta, percore = preprocess(inputs['edge_index'])
    has_cb = bool(np.any(np.asarray(inputs['conv_b'], dtype=np.float32)))
    nc = build_kernel(meta, has_cb)
    in_maps = make_in_maps(inputs, meta, percore)
    res = run_bass_kernel_spmd(nc, in_maps, core_ids=list(range(NCORES)),
                               trace=trace)
    return res


def kernel(**inputs):
    res = run(inputs)
    return np.asarray(res.results[0]['out'], dtype=np.float32)
